# revision 1
# baseline (speedup 1.0000x reference)
"""Causal single-head attention on 8 Trainium2 NeuronCores.

Problem: x [8, 2048, 1024] f32, Wq/Wk/Wv [1024, 1024] f32.
  q,k,v = x @ W*;  out = softmax(mask(q k^T)/sqrt(1024)) @ v

Sharding: data-parallel over batch — one batch element per core, weights
replicated. Each core runs an identical single-core program (SPMD, no
collectives).

Per-core kernel design (S=2048 seq, D=1024 model dim, P=128 partitions):
  Phase 0: x^T [D, S] built via PE transposes (needed since matmul contracts
           over the partition dim).
  Phase 1: Q-pass then K-pass (dc-outer / ib-inner so each weight load feeds
           4 matmuls), spilled to per-i-block DRAM scratch tensors on the
           ACT DMA queue; V = x^T-chunks @ Wv stays SBUF-resident
           [P, 16, D].
  Phase 2: per 512-wide i-block (transposed-scores flash attention):
           S^T[j,i] tiles = K^T-chunk.T @ Q^T-chunk (accumulate over e);
           diagonal tiles are narrowed to their unmasked i-range and get an
           additive causal mask before exp on ACT (scale 1/sqrt(D) fused)
           -> P~ tiles (already transposed for the AV matmul). No max
           subtraction: scores are ~N(0,1) so exp is safe in fp32.
           out[i,e] = P~-tile.T @ V-tile accumulated over j, with the
           softmax denominator folded in as a third matmul against a ones
           column reusing the loaded P~ weights; fully-masked (j,i-sub)
           pairs are skipped; final 1/l scale on DVE.

All matmul inputs are float32r (TF32-class, full PE speed at free dim
>= 256; fp32 would be 4x slower); accumulation is fp32 in PSUM. Measured
end-to-end fro rel err vs fp32 CPU reference: 2.96e-4. Device time
~390-435us per 8-core SPMD execution (cost model predicts 405us; exact
causal-compute roofline is 273us at 78.6 TF/s).
"""

import numpy as np

import concourse.bass as bass  # noqa: F401  (engine types referenced via nc)
import concourse.mybir as mybir
import concourse.tile as tile
from concourse import bacc
from concourse.bass_utils import run_bass_kernel_spmd
from concourse.masks import make_identity

F32 = mybir.dt.float32
F32R = mybir.dt.float32r

B = 8
S = 2048
D = 1024
P = 128
EC = D // P          # 8 e/d chunks of 128
IB = 512             # i-block width
NIB = S // IB        # 4
NJT = S // P         # 16 j-tiles
SCALE = 1.0 / 32.0   # 1/sqrt(D)
NEG = -1.0e9

_CACHE: dict = {}


def _build(reps: int = 1):
    """reps > 1 repeats the whole body (for device-time slope measurement)."""
    nc = bacc.Bacc("TRN2", target_bir_lowering=False, debug=False)
    x_d = nc.dram_tensor("x", [S, D], F32, kind="ExternalInput")
    wq_d = nc.dram_tensor("Wq", [D, D], F32R, kind="ExternalInput")
    wk_d = nc.dram_tensor("Wk", [D, D], F32R, kind="ExternalInput")
    wv_d = nc.dram_tensor("Wv", [D, D], F32R, kind="ExternalInput")
    out_d = nc.dram_tensor("out", [S, D], F32, kind="ExternalOutput")

    Exp = mybir.ActivationFunctionType.Exp

    with tile.TileContext(nc) as tc:
        for _rep in range(reps):
            _emit_body(nc, tc, x_d, wq_d, wk_d, wv_d, out_d, Exp)
    nc.compile()
    return nc


def _emit_body(nc, tc, x_d, wq_d, wk_d, wv_d, out_d, Exp):
    if True:  # keep indentation of the original body
        with (
            tc.tile_pool(name="persist", bufs=1) as pers,
            tc.tile_pool(name="dram", bufs=1, space="DRAM") as dpool,
        ):
            v_sb = pers.tile([P, NJT, D], F32R, tag="v")
            bigmask = pers.tile([P, 2 * IB], F32, tag="bigmask")
            # fp32r matmuls need even free sizes -> 2-wide ones/l columns
            ones_sb = pers.tile([P, 2], F32R, tag="ones")
            ones_f32 = pers.tile([P, 2], F32, tag="ones32")
            # per-i-block scratch tensors: phase-2 readers of block b then
            # only depend on block-b spill writes, not the whole phase
            qt_ds = [dpool.tile([P, EC, IB], F32R, tag=f"qt{b}", name=f"qt{b}")
                     for b in range(NIB)]
            kt_ds = [dpool.tile([P, EC, IB], F32R, tag=f"kt{b}", name=f"kt{b}")
                     for b in range(NIB)]

            # bigmask[p, c] = 0 iff p <= c - IB else NEG  (additive causal mask;
            # slice [IB - r : 2*IB - r] gives "keep iff p <= col - r")
            nc.gpsimd.memset(bigmask[:], 0.0)
            nc.gpsimd.affine_select(
                out=bigmask[:],
                in_=bigmask[:],
                compare_op=mybir.AluOpType.is_ge,
                fill=NEG,
                base=-IB,
                pattern=[[1, 2 * IB]],
                channel_multiplier=-1,
            )
            nc.vector.memset(ones_f32[:], 1.0)
            nc.vector.tensor_copy(out=ones_sb[:], in_=ones_f32[:])

            # ---------- Phase 0: x^T via PE transposes ----------
            xt_cm = tc.tile_pool(name="xtp", bufs=1)
            xt_pool = xt_cm.__enter__()  # closed manually after phase 1
            xt_sb = xt_pool.tile([P, EC, S], F32R, tag="xt")
            with (
                tc.tile_pool(name="ph0", bufs=1) as p0,
                tc.tile_pool(name="ph0_psum", bufs=1, space="PSUM") as p0p,
            ):
                ident = p0.tile([P, P], F32, tag="ident")
                make_identity(nc, ident[:])
                for it in range(S // P):
                    x_in = p0.tile([P, D], F32, tag="xin", bufs=4)
                    nc.sync.dma_start(x_in[:], x_d.ap()[it * P:(it + 1) * P, :])
                    for dc in range(EC):
                        ps_t = p0p.tile([P, P], F32, tag="pst", bufs=4)
                        nc.tensor.transpose(
                            ps_t[:], x_in[:, dc * P:(dc + 1) * P], ident[:]
                        )
                        nc.vector.tensor_copy(
                            out=xt_sb[:, dc, it * P:(it + 1) * P], in_=ps_t[:]
                        )

            # ---------- Phase 1: projections ----------
            # QK: dc-outer / ib-inner so each W weight-load feeds 4 matmuls.
            p1v_cm = tc.tile_pool(name="ph1v", bufs=1)
            p1v = p1v_cm.__enter__()
            wv_sb = p1v.tile([P, EC, D], F32R, tag="wv")
            # prefetch Wv early so the V phase doesn't stall behind the
            # qt/kt spill writes in the DMA queues
            nc.sync.dma_start(
                wv_sb[:], wv_d.ap().rearrange("(dc p) e -> p dc e", p=P)
            )
            p1w_cm = tc.tile_pool(name="ph1w", bufs=1)
            p1w = p1w_cm.__enter__()
            for w_d, t_ds, wtag in ((wq_d, qt_ds, "wq"), (wk_d, kt_ds, "wk")):
                with (
                    tc.tile_pool(name=f"ph1{wtag}", bufs=1) as p1,
                    tc.tile_pool(name=f"ph1{wtag}_psum", bufs=1,
                                 space="PSUM") as p1p,
                ):
                    w_r = w_d.ap().rearrange("(dc p) e -> p dc e", p=P)
                    for ec in range(EC):
                        w_sb = p1w.tile([P, EC, P], F32R, tag=wtag, bufs=2,
                                        name=f"{wtag}_sb")
                        nc.sync.dma_start(w_sb[:], w_r[:, :, ec * P:(ec + 1) * P])
                        ps_q = [
                            p1p.tile([P, IB], F32, tag=f"ps{wtag}{ib}",
                                     name=f"ps_{wtag}{ib}", bufs=2)
                            for ib in range(NIB)
                        ]
                        for dc in range(EC):
                            for ib in range(NIB):
                                isl = slice(ib * IB, (ib + 1) * IB)
                                nc.tensor.matmul(
                                    ps_q[ib][:], lhsT=w_sb[:, dc],
                                    rhs=xt_sb[:, dc, isl],
                                    start=dc == 0, stop=dc == EC - 1,
                                )
                        for ib in range(NIB):
                            st_q = p1.tile([P, IB], F32R, tag=f"st{wtag}",
                                           bufs=3, name=f"st_{wtag}")
                            nc.vector.tensor_copy(out=st_q[:], in_=ps_q[ib][:])
                            nc.scalar.dma_start(t_ds[ib][:, ec, :], st_q[:])
            p1w_cm.__exit__(None, None, None)

            with tc.tile_pool(name="ph1v_psum", bufs=1, space="PSUM") as p1vp:
                for jc in range(NJT):
                    jsl = slice(jc * P, (jc + 1) * P)
                    ps_v0 = p1vp.tile([P, 512], F32, tag="psv0", bufs=2)
                    ps_v1 = p1vp.tile([P, 512], F32, tag="psv1", bufs=2)
                    for dc in range(EC):
                        nc.tensor.matmul(
                            ps_v0[:], lhsT=xt_sb[:, dc, jsl], rhs=wv_sb[:, dc, 0:512],
                            start=dc == 0, stop=dc == EC - 1,
                        )
                        nc.tensor.matmul(
                            ps_v1[:], lhsT=xt_sb[:, dc, jsl], rhs=wv_sb[:, dc, 512:1024],
                            start=dc == 0, stop=dc == EC - 1,
                        )
                    nc.vector.tensor_copy(out=v_sb[:, jc, 0:512], in_=ps_v0[:])
                    nc.vector.tensor_copy(out=v_sb[:, jc, 512:1024], in_=ps_v1[:])

            p1v_cm.__exit__(None, None, None)
            xt_cm.__exit__(None, None, None)

            # ---------- Phase 2: attention ----------
            with (
                tc.tile_pool(name="ph2", bufs=1) as p2,
                tc.tile_pool(name="ph2_psum", bufs=1, space="PSUM") as p2p,
            ):
                for b in range(NIB):
                    qt_b = p2.tile([P, EC, IB], F32R, tag="qtb", bufs=2)
                    nc.sync.dma_start(qt_b[:], qt_ds[b][:])
                    # ptiles[jt] = (tile, r): tile covers i_rel in [r, 512)
                    ptiles = []
                    for jc in range(b + 1):
                        kt_b = p2.tile([P, EC, IB], F32R, tag="ktb", bufs=2)
                        nc.sync.dma_start(kt_b[:], kt_ds[jc][:])
                        for js in range(4):
                            jt = jc * 4 + js
                            r = max(jt * P - b * IB, 0)
                            w = IB - r  # narrowed width for diagonal tiles
                            ps_s = p2p.tile([P, IB], F32, tag="pss", bufs=2)
                            for ec in range(EC):
                                nc.tensor.matmul(
                                    ps_s[:, :w],
                                    lhsT=kt_b[:, ec, js * P:(js + 1) * P],
                                    rhs=qt_b[:, ec, r:IB],
                                    start=ec == 0, stop=ec == EC - 1,
                                )
                            if r > 0 or jt * P == b * IB:
                                # diagonal tile: additive causal mask
                                # (keep iff p <= col')
                                nc.vector.tensor_add(
                                    ps_s[:, :w], ps_s[:, :w],
                                    bigmask[:, IB:IB + w],
                                )
                            pt = p2.tile([P, IB], F32R, tag="pt", bufs=24)
                            nc.scalar.activation(pt[:, :w], ps_s[:, :w], Exp,
                                                 scale=SCALE)
                            ptiles.append((pt, r))

                    for isub in range(4):
                        i0 = b * IB + isub * P
                        # j-tiles with any unmasked entry for this i-subtile
                        ks = [k for k, (_, r) in enumerate(ptiles)
                              if r <= isub * P]
                        ps_l = p2p.tile([P, 2], F32, tag="psl", bufs=2)
                        ps_o0 = p2p.tile([P, 512], F32, tag="po0", bufs=2)
                        ps_o1 = p2p.tile([P, 512], F32, tag="po1", bufs=2)
                        for n, k in enumerate(ks):
                            pt, r = ptiles[k]
                            lhsT = pt[:, isub * P - r:(isub + 1) * P - r]
                            first, last = n == 0, n == len(ks) - 1
                            nc.tensor.matmul(
                                ps_o0[:], lhsT=lhsT, rhs=v_sb[:, k, 0:512],
                                start=first, stop=last,
                            )
                            nc.tensor.matmul(
                                ps_o1[:], lhsT=lhsT, rhs=v_sb[:, k, 512:1024],
                                start=first, stop=last,
                            )
                            nc.tensor.matmul(
                                ps_l[:], lhsT=lhsT, rhs=ones_sb[:],
                                start=first, stop=last,
                            )
                        recip = p2.tile([P, 2], F32, tag="recip", bufs=2)
                        nc.vector.reciprocal(recip[:], ps_l[:])
                        st0 = p2.tile([P, 512], F32, tag="st0", bufs=2)
                        st1 = p2.tile([P, 512], F32, tag="st1", bufs=2)
                        nc.vector.tensor_scalar_mul(st0[:], ps_o0[:], recip[:, 0:1])
                        nc.vector.tensor_scalar_mul(st1[:], ps_o1[:], recip[:, 0:1])
                        nc.sync.dma_start(out_d.ap()[i0:i0 + P, 0:512], st0[:])
                        nc.sync.dma_start(out_d.ap()[i0:i0 + P, 512:1024], st1[:])


def kernel(x: np.ndarray, Wq: np.ndarray, Wk: np.ndarray, Wv: np.ndarray) -> np.ndarray:
    if "nc" not in _CACHE:
        _CACHE["nc"] = _build()
    nc = _CACHE["nc"]

    x = np.ascontiguousarray(np.asarray(x, dtype=np.float32))
    Wq = np.ascontiguousarray(np.asarray(Wq, dtype=np.float32))
    Wk = np.ascontiguousarray(np.asarray(Wk, dtype=np.float32))
    Wv = np.ascontiguousarray(np.asarray(Wv, dtype=np.float32))

    in_maps = [
        {"x": x[c], "Wq": Wq, "Wk": Wk, "Wv": Wv} for c in range(B)
    ]
    res = run_bass_kernel_spmd(nc, in_maps, core_ids=list(range(B)))
    return np.stack([res.results[c]["out"] for c in range(B)], axis=0)


def _selftest():
    """Smoke test against a numpy fp64 reference on random data."""
    rng = np.random.default_rng(0)
    x = rng.standard_normal((B, S, D), dtype=np.float32)
    w = [rng.standard_normal((D, D), dtype=np.float32).astype(np.float32) / 32.0
         for _ in range(3)]
    out = kernel(x, *w)
    x64 = x.astype(np.float64)
    q, k, v = (x64 @ wi.astype(np.float64) for wi in w)
    s = np.einsum("bqe,bke->bqk", q, k) / 32.0
    mask = np.triu(np.ones((S, S), dtype=bool), k=1)
    s = np.where(mask[None], -np.inf, s)
    s -= s.max(-1, keepdims=True)
    p = np.exp(s)
    p /= p.sum(-1, keepdims=True)
    ref = np.einsum("bqk,bke->bqe", p, v)
    fro = np.linalg.norm(out - ref) / np.linalg.norm(ref)
    print(f"selftest rel err: {fro:.3e}")
    return fro


if __name__ == "__main__":
    _selftest()



# revision 5
# speedup vs baseline: 1.1465x; 1.1465x over previous
"""Causal single-head attention on 8 Trainium2 NeuronCores.

Problem: x [8, 2048, 1024] f32, Wq/Wk/Wv [1024, 1024] f32.
  q,k,v = x @ W*;  out = softmax(mask(q k^T)/sqrt(1024)) @ v

Sharding: data-parallel over batch — one batch element per core, weights
replicated. Each core runs an identical single-core program (SPMD, no
collectives).

Per-core kernel design (S=2048 seq, D=1024 model dim, P=128 partitions),
fully fused per 512-row i-block, everything SBUF-resident in bf16 (no
DRAM scratch):
  Prelude: Wq/Wk/Wv streamed in fp32, converted to bf16 on Pool.
  Per block b (4 blocks of 512 rows):
    x rows -> bf16 (Pool) -> PE transposes (bf16, 1 cyc/row) -> xt_b
    Q_b/K_b/V_b projections (bf16 matmuls, fp32 PSUM accum); K^T and V
    accumulate into persistent SBUF tiles, qt_b is per-block.
    Transposed-scores flash attention against j-chunks 0..b: S^T tiles =
    K^T-chunk.T @ Q^T-chunk; diagonal tiles narrowed + additive causal
    mask; exp on ACT (scale 1/32 fused) -> P~ bf16 tiles (already
    transposed for AV). out = P~.T @ V accumulated over j with the
    softmax denominator as a third matmul against a ones column; final
    1/l scale on DVE.
bf16 operands keep end-to-end fro rel err ~3e-3 (vs 2e-2 gate); fp32
would cost 4x PE throughput, fp8 costs ~3e-2 error (measured) - too
much. PSUM accumulation is fp32 throughout.
"""

import numpy as np

import concourse.bass as bass  # noqa: F401
import concourse.mybir as mybir
import concourse.tile as tile
from concourse import bacc
from concourse.bass_utils import run_bass_kernel_spmd
from concourse.masks import make_identity

F32 = mybir.dt.float32
BF16 = mybir.dt.bfloat16

B = 8
S = 2048
D = 1024
P = 128
EC = D // P          # 8 e/d chunks of 128
IB = 512             # i-block width
NIB = S // IB        # 4
NJT = S // P         # 16 j-tiles
SCALE = 1.0 / 32.0   # 1/sqrt(D)
NEG = -1.0e9

_CACHE: dict = {}


def _build(reps: int = 1):
    """reps > 1 repeats the whole body (for device-time slope measurement)."""
    nc = bacc.Bacc("TRN2", target_bir_lowering=False, debug=False)
    x_d = nc.dram_tensor("x", [S, D], F32, kind="ExternalInput")
    wq_d = nc.dram_tensor("Wq", [D, D], F32, kind="ExternalInput")
    wk_d = nc.dram_tensor("Wk", [D, D], F32, kind="ExternalInput")
    wv_d = nc.dram_tensor("Wv", [D, D], F32, kind="ExternalInput")
    out_d = nc.dram_tensor("out", [S, D], F32, kind="ExternalOutput")

    Exp = mybir.ActivationFunctionType.Exp

    with tile.TileContext(nc) as tc:
        for _rep in range(reps):
            _emit_body(nc, tc, x_d, wq_d, wk_d, wv_d, out_d, Exp)
    nc.compile()
    return nc


def _emit_body(nc, tc, x_d, wq_d, wk_d, wv_d, out_d, Exp):
    with (
        tc.tile_pool(name="persist", bufs=1) as pers,
        tc.tile_pool(name="work", bufs=1) as wk,
        tc.tile_pool(name="psum", bufs=1, space="PSUM") as pp,
    ):
        # ---- persistent bf16 tensors ----
        w_sbs = {
            t: pers.tile([P, EC, D], BF16, tag=t, name=t)
            for t in ("wq", "wk", "wv")
        }
        kt_blocks = [pers.tile([P, EC, IB], BF16, tag=f"ktb{b}",
                               name=f"ktb{b}") for b in range(NIB)]
        v_blocks = [pers.tile([P, 4, D], BF16, tag=f"vb{b}",
                              name=f"vb{b}") for b in range(NIB)]
        bigmask = pers.tile([P, 2 * IB], F32, tag="bigmask")
        ones_sb = pers.tile([P, 2], BF16, tag="ones")
        ident = pers.tile([P, P], BF16, tag="ident")

        # bigmask[p, c] = 0 iff p <= c - IB else NEG (additive causal mask;
        # slice [IB : IB + w] gives "keep iff p <= col")
        nc.gpsimd.memset(bigmask[:], 0.0)
        nc.gpsimd.affine_select(
            out=bigmask[:],
            in_=bigmask[:],
            compare_op=mybir.AluOpType.is_ge,
            fill=NEG,
            base=-IB,
            pattern=[[1, 2 * IB]],
            channel_multiplier=-1,
        )
        nc.vector.memset(ones_sb[:], 1.0)
        make_identity(nc, ident[:])

        # ---- weights: stream fp32, convert to bf16 on Pool ----
        # [d, e] -> [p, dc, e]; per ec-slice of 128 output cols
        for w_d, wtag in ((wq_d, "wq"), (wk_d, "wk"), (wv_d, "wv")):
            w_r = w_d.ap().rearrange("(dc p) e -> p dc e", p=P)
            for ec in range(EC):
                esl = slice(ec * P, (ec + 1) * P)
                w_st = wk.tile([P, EC, P], F32, tag="w_st", bufs=3)
                nc.scalar.dma_start(w_st[:], w_r[:, :, esl])
                nc.gpsimd.tensor_copy(out=w_sbs[wtag][:, :, esl], in_=w_st[:])

        for b in range(NIB):
            # ---- x rows for block b: load fp32, bf16-convert, transpose ----
            xt_b = wk.tile([P, EC, IB], BF16, tag="xt", bufs=2)
            for itr in range(IB // P):
                it = b * (IB // P) + itr
                x_in = wk.tile([P, D], F32, tag="xin", bufs=3)
                nc.sync.dma_start(x_in[:], x_d.ap()[it * P:(it + 1) * P, :])
                xb = wk.tile([P, D], BF16, tag="xb", bufs=2)
                nc.gpsimd.tensor_copy(out=xb[:], in_=x_in[:])
                for dh in range(2):  # two psum tiles of 4 transposes each
                    ps_t = pp.tile([P, 4, P], BF16, tag="pst", bufs=2)
                    for dq in range(4):
                        dc = dh * 4 + dq
                        nc.tensor.transpose(
                            ps_t[:, dq], xb[:, dc * P:(dc + 1) * P], ident[:]
                        )
                    nc.vector.tensor_copy(
                        out=xt_b[:, dh * 4:(dh + 1) * 4,
                                 itr * P:(itr + 1) * P],
                        in_=ps_t[:],
                    )

            # ---- projections for block b ----
            qt_b = wk.tile([P, EC, IB], BF16, tag="qt", bufs=2)
            for wtag, dst in (("wq", None), ("wk", None)):
                w_sb = w_sbs[wtag]
                for ec in range(EC):
                    ps = pp.tile([P, IB], F32, tag="ps512", bufs=2)
                    for dc in range(EC):
                        nc.tensor.matmul(
                            ps[:], lhsT=w_sb[:, dc, ec * P:(ec + 1) * P],
                            rhs=xt_b[:, dc, :],
                            start=dc == 0, stop=dc == EC - 1,
                        )
                    out_sl = (qt_b if wtag == "wq" else kt_blocks[b])
                    nc.vector.tensor_copy(out=out_sl[:, ec, :], in_=ps[:])
            wv_sb = w_sbs["wv"]
            for js in range(4):
                jsl = slice(js * P, (js + 1) * P)
                # h-inner so each stationary xt slice feeds both e-halves
                ps_h = [pp.tile([P, IB], F32, tag="ps512", bufs=2,
                                name=f"ps_v{h}") for h in range(2)]
                for dc in range(EC):
                    for h in range(2):
                        nc.tensor.matmul(
                            ps_h[h][:], lhsT=xt_b[:, dc, jsl],
                            rhs=wv_sb[:, dc, h * IB:(h + 1) * IB],
                            start=dc == 0, stop=dc == EC - 1,
                        )
                for h in range(2):
                    nc.vector.tensor_copy(
                        out=v_blocks[b][:, js, h * IB:(h + 1) * IB],
                        in_=ps_h[h][:]
                    )

            # ---- attention for block b (transposed-scores flash) ----
            # ptiles[jt] = (tile, r): tile covers i_rel in [r, 512)
            ptiles = []
            for jc in range(b + 1):
                kt_c = kt_blocks[jc]
                for js in range(4):
                    jt = jc * 4 + js
                    r = max(jt * P - b * IB, 0)
                    w = IB - r  # narrowed width for diagonal tiles
                    ps_s = pp.tile([P, IB], F32, tag="ps512", bufs=2)
                    for ec in range(EC):
                        nc.tensor.matmul(
                            ps_s[:, :w],
                            lhsT=kt_c[:, ec, js * P:(js + 1) * P],
                            rhs=qt_b[:, ec, r:IB],
                            start=ec == 0, stop=ec == EC - 1,
                        )
                    if jc == b:
                        # diagonal tile: additive causal mask (keep iff
                        # p <= col')
                        nc.vector.tensor_add(
                            ps_s[:, :w], ps_s[:, :w], bigmask[:, IB:IB + w],
                        )
                    pt = wk.tile([P, IB], BF16, tag="pt", bufs=20)
                    nc.scalar.activation(pt[:, :w], ps_s[:, :w], Exp,
                                         scale=SCALE)
                    ptiles.append((pt, r))

            for isub in range(4):
                i0 = b * IB + isub * P
                # j-tiles with any unmasked entry for this i-subtile
                ks = [k for k, (_, r) in enumerate(ptiles) if r <= isub * P]
                ps_l = pp.tile([P, 2], F32, tag="psl", bufs=1)
                ps_o0 = pp.tile([P, IB], F32, tag="po0", bufs=2)
                ps_o1 = pp.tile([P, IB], F32, tag="po1", bufs=1)
                for n, k in enumerate(ks):
                    pt, r = ptiles[k]
                    lhsT = pt[:, isub * P - r:(isub + 1) * P - r]
                    v_t = v_blocks[k // 4]
                    first, last = n == 0, n == len(ks) - 1
                    nc.tensor.matmul(
                        ps_o0[:], lhsT=lhsT, rhs=v_t[:, k % 4, 0:IB],
                        start=first, stop=last,
                    )
                    nc.tensor.matmul(
                        ps_o1[:], lhsT=lhsT, rhs=v_t[:, k % 4, IB:D],
                        start=first, stop=last,
                    )
                    nc.tensor.matmul(
                        ps_l[:], lhsT=lhsT, rhs=ones_sb[:],
                        start=first, stop=last,
                    )
                recip = wk.tile([P, 2], F32, tag="recip", bufs=2)
                nc.vector.reciprocal(recip[:], ps_l[:])
                st0 = wk.tile([P, IB], F32, tag="st0", bufs=2)
                st1 = wk.tile([P, IB], F32, tag="st1", bufs=2)
                nc.vector.tensor_scalar_mul(st0[:], ps_o0[:], recip[:, 0:1])
                nc.vector.tensor_scalar_mul(st1[:], ps_o1[:], recip[:, 0:1])
                nc.sync.dma_start(out_d.ap()[i0:i0 + P, 0:IB], st0[:])
                nc.sync.dma_start(out_d.ap()[i0:i0 + P, IB:D], st1[:])


def kernel(x: np.ndarray, Wq: np.ndarray, Wk: np.ndarray, Wv: np.ndarray) -> np.ndarray:
    if "nc" not in _CACHE:
        _CACHE["nc"] = _build()
    nc = _CACHE["nc"]

    x = np.ascontiguousarray(np.asarray(x, dtype=np.float32))
    Wq = np.ascontiguousarray(np.asarray(Wq, dtype=np.float32))
    Wk = np.ascontiguousarray(np.asarray(Wk, dtype=np.float32))
    Wv = np.ascontiguousarray(np.asarray(Wv, dtype=np.float32))

    in_maps = [
        {"x": x[c], "Wq": Wq, "Wk": Wk, "Wv": Wv} for c in range(B)
    ]
    res = run_bass_kernel_spmd(nc, in_maps, core_ids=list(range(B)))
    return np.stack([res.results[c]["out"] for c in range(B)], axis=0)


def _selftest():
    """Smoke test against a numpy fp64 reference on random data."""
    rng = np.random.default_rng(0)
    x = rng.standard_normal((B, S, D), dtype=np.float32)
    w = [rng.standard_normal((D, D), dtype=np.float32).astype(np.float32) / 32.0
         for _ in range(3)]
    out = kernel(x, *w)
    x64 = x.astype(np.float64)
    q, k, v = (x64 @ wi.astype(np.float64) for wi in w)
    s = np.einsum("bqe,bke->bqk", q, k) / 32.0
    mask = np.triu(np.ones((S, S), dtype=bool), k=1)
    s = np.where(mask[None], -np.inf, s)
    s -= s.max(-1, keepdims=True)
    p = np.exp(s)
    p /= p.sum(-1, keepdims=True)
    ref = np.einsum("bqk,bke->bqe", p, v)
    fro = np.linalg.norm(out - ref) / np.linalg.norm(ref)
    print(f"selftest rel err: {fro:.3e}")
    return fro


if __name__ == "__main__":
    _selftest()


# revision 10
# speedup vs baseline: 1.3437x; 1.1720x over previous
"""Causal single-head attention on 8 Trainium2 NeuronCores.

Problem: x [8, 2048, 1024] f32, Wq/Wk/Wv [1024, 1024] f32.
  q,k,v = x @ W*;  out = softmax(mask(q k^T)/sqrt(1024)) @ v

Sharding: data-parallel over batch — one batch element per core, weights
replicated. Each core runs an identical single-core program (SPMD, no
collectives).

Per-core kernel design (S=2048 seq, D=1024 model dim, P=128 partitions),
fully fused per 512-row i-block, everything SBUF-resident in bf16 (no
DRAM scratch):
  Prelude: Wq/Wk/Wv streamed in fp32, converted to bf16 on Pool.
  Per block b (4 blocks of 512 rows):
    x rows -> bf16 (Pool) -> PE transposes (bf16, 1 cyc/row) -> xt_b
    Q_b/K_b/V_b projections (bf16 matmuls, fp32 PSUM accum); K^T and V
    accumulate into persistent SBUF tiles, qt_b is per-block.
    Transposed-scores flash attention against j-chunks 0..b: S^T tiles =
    K^T-chunk.T @ Q^T-chunk; diagonal tiles narrowed + additive causal
    mask; exp on ACT (scale 1/32 fused) -> P~ bf16 tiles (already
    transposed for AV). out = P~.T @ V accumulated over j with the
    softmax denominator as a third matmul against a ones column; final
    1/l scale on DVE.
bf16 operands keep end-to-end fro rel err ~3e-3 (vs 2e-2 gate); fp32
would cost 4x PE throughput, fp8 costs ~3e-2 error (measured) - too
much. PSUM accumulation is fp32 throughout.
"""

import numpy as np

import concourse.bass as bass  # noqa: F401
import concourse.mybir as mybir
import concourse.tile as tile
from concourse import bacc
from concourse.bass_utils import run_bass_kernel_spmd
from concourse.masks import make_identity

F32 = mybir.dt.float32
BF16 = mybir.dt.bfloat16

B = 8
S = 2048
D = 1024
P = 128
EC = D // P          # 8 e/d chunks of 128
IB = 512             # i-block width
NIB = S // IB        # 4
NJT = S // P         # 16 j-tiles
SCALE = 1.0 / 32.0   # 1/sqrt(D)
NEG = -1.0e9

_CACHE: dict = {}


def _build(reps: int = 1):
    """reps > 1 repeats the whole body (for device-time slope measurement)."""
    nc = bacc.Bacc("TRN2", target_bir_lowering=False, debug=False)
    x_d = nc.dram_tensor("x", [S, D], F32, kind="ExternalInput")
    wq_d = nc.dram_tensor("Wq", [D, D], F32, kind="ExternalInput")
    wk_d = nc.dram_tensor("Wk", [D, D], F32, kind="ExternalInput")
    wv_d = nc.dram_tensor("Wv", [D, D], F32, kind="ExternalInput")
    out_d = nc.dram_tensor("out", [S, D], F32, kind="ExternalOutput")

    Exp = mybir.ActivationFunctionType.Exp

    with tile.TileContext(nc) as tc:
        for _rep in range(reps):
            _emit_body(nc, tc, x_d, wq_d, wk_d, wv_d, out_d, Exp)
    nc.compile()
    return nc


def _emit_body(nc, tc, x_d, wq_d, wk_d, wv_d, out_d, Exp):
    with (
        tc.tile_pool(name="persist", bufs=1) as pers,
        tc.tile_pool(name="work", bufs=1) as wk,
        tc.tile_pool(name="psum", bufs=1, space="PSUM") as pp,
    ):
        # ---- persistent bf16 tensors ----
        w_sbs = {
            t: pers.tile([P, EC, D], BF16, tag=t, name=t)
            for t in ("wq", "wk", "wv")
        }
        kt_blocks = [pers.tile([P, EC, IB], BF16, tag=f"ktb{b}",
                               name=f"ktb{b}") for b in range(NIB)]
        v_blocks = [pers.tile([P, 4, D], BF16, tag=f"vb{b}",
                              name=f"vb{b}") for b in range(NIB)]
        bigmask = pers.tile([P, 2 * IB], F32, tag="bigmask")
        ones_sb = pers.tile([P, 2], BF16, tag="ones")
        ident = pers.tile([P, P], BF16, tag="ident")

        # bigmask[p, c] = 0 iff p <= c - IB else NEG (additive causal mask;
        # slice [IB : IB + w] gives "keep iff p <= col")
        nc.gpsimd.memset(bigmask[:], 0.0)
        nc.gpsimd.affine_select(
            out=bigmask[:],
            in_=bigmask[:],
            compare_op=mybir.AluOpType.is_ge,
            fill=NEG,
            base=-IB,
            pattern=[[1, 2 * IB]],
            channel_multiplier=-1,
        )
        nc.vector.memset(ones_sb[:], 1.0)
        make_identity(nc, ident[:])

        # ---- weights: stream fp32, convert to bf16 on Pool ----
        # [d, e] -> [p, dc, e]; per ec-slice of 128 output cols
        for w_d, wtag in ((wq_d, "wq"), (wk_d, "wk"), (wv_d, "wv")):
            w_r = w_d.ap().rearrange("(dc p) e -> p dc e", p=P)
            for ec in range(EC):
                esl = slice(ec * P, (ec + 1) * P)
                w_st = wk.tile([P, EC, P], F32, tag="w_st", bufs=3)
                nc.scalar.dma_start(w_st[:], w_r[:, :, esl])
                nc.gpsimd.tensor_copy(out=w_sbs[wtag][:, :, esl], in_=w_st[:])

        for b in range(NIB):
            # ---- x rows for block b: load fp32, bf16-convert, transpose ----
            xt_b = wk.tile([P, EC, IB], BF16, tag="xt", bufs=2)
            for itr in range(IB // P):
                it = b * (IB // P) + itr
                x_in = wk.tile([P, D], F32, tag="xin", bufs=3)
                xb = wk.tile([P, D], BF16, tag="xb", bufs=2)
                for h in range(2):  # halves so transposes start sooner
                    hsl = slice(h * IB, (h + 1) * IB)
                    nc.sync.dma_start(x_in[:, hsl],
                                      x_d.ap()[it * P:(it + 1) * P, hsl])
                    nc.gpsimd.tensor_copy(out=xb[:, hsl], in_=x_in[:, hsl])
                for dh in range(2):  # two psum tiles of 4 transposes each
                    ps_t = pp.tile([P, 4, P], BF16, tag="pst", bufs=2)
                    for dq in range(4):
                        dc = dh * 4 + dq
                        nc.tensor.transpose(
                            ps_t[:, dq], xb[:, dc * P:(dc + 1) * P], ident[:]
                        )
                    nc.vector.tensor_copy(
                        out=xt_b[:, dh * 4:(dh + 1) * 4,
                                 itr * P:(itr + 1) * P],
                        in_=ps_t[:],
                    )

            # ---- projections for block b ----
            qt_b = wk.tile([P, EC, IB], BF16, tag="qt", bufs=2)
            for wtag, dst in (("wq", None), ("wk", None)):
                w_sb = w_sbs[wtag]
                for ec in range(EC):
                    ps = pp.tile([P, IB], F32, tag="ps512", bufs=2)
                    for dc in range(EC):
                        nc.tensor.matmul(
                            ps[:], lhsT=w_sb[:, dc, ec * P:(ec + 1) * P],
                            rhs=xt_b[:, dc, :],
                            start=dc == 0, stop=dc == EC - 1,
                        )
                    out_sl = (qt_b if wtag == "wq" else kt_blocks[b])
                    nc.vector.tensor_copy(out=out_sl[:, ec, :], in_=ps[:])
            wv_sb = w_sbs["wv"]
            for js in range(4):
                jsl = slice(js * P, (js + 1) * P)
                # h-inner so each stationary xt slice feeds both e-halves
                ps_h = [pp.tile([P, IB], F32, tag="ps512", bufs=2,
                                name=f"ps_v{h}") for h in range(2)]
                for dc in range(EC):
                    for h in range(2):
                        nc.tensor.matmul(
                            ps_h[h][:], lhsT=xt_b[:, dc, jsl],
                            rhs=wv_sb[:, dc, h * IB:(h + 1) * IB],
                            start=dc == 0, stop=dc == EC - 1,
                        )
                for h in range(2):
                    nc.vector.tensor_copy(
                        out=v_blocks[b][:, js, h * IB:(h + 1) * IB],
                        in_=ps_h[h][:]
                    )

            # ---- attention for block b (transposed-scores flash) ----
            # ptiles[jt] = (tile, r): tile covers i_rel in [r, 512)
            ptiles = []
            for jc in range(b + 1):
                kt_c = kt_blocks[jc]
                for js in range(4):
                    jt = jc * 4 + js
                    r = max(jt * P - b * IB, 0)
                    w = IB - r  # narrowed width for diagonal tiles
                    ps_s = pp.tile([P, IB], F32, tag="ps512", bufs=2)
                    for ec in range(EC):
                        nc.tensor.matmul(
                            ps_s[:, :w],
                            lhsT=kt_c[:, ec, js * P:(js + 1) * P],
                            rhs=qt_b[:, ec, r:IB],
                            start=ec == 0, stop=ec == EC - 1,
                        )
                    if jc == b:
                        # diagonal tile: additive causal mask (keep iff
                        # p <= col')
                        nc.vector.tensor_add(
                            ps_s[:, :w], ps_s[:, :w], bigmask[:, IB:IB + w],
                        )
                    pt = wk.tile([P, IB], BF16, tag="pt", bufs=20)
                    nc.scalar.activation(pt[:, :w], ps_s[:, :w], Exp,
                                         scale=SCALE)
                    ptiles.append((pt, r))

            for isub in range(4):
                i0 = b * IB + isub * P
                # j-tiles with any unmasked entry for this i-subtile
                ks = [k for k, (_, r) in enumerate(ptiles) if r <= isub * P]
                ps_l = pp.tile([P, 2], F32, tag="psl", bufs=1)
                ps_o0 = pp.tile([P, IB], F32, tag="po0", bufs=2)
                ps_o1 = pp.tile([P, IB], F32, tag="po1", bufs=1)
                for n, k in enumerate(ks):
                    pt, r = ptiles[k]
                    lhsT = pt[:, isub * P - r:(isub + 1) * P - r]
                    v_t = v_blocks[k // 4]
                    first, last = n == 0, n == len(ks) - 1
                    # psl first: its single buffer frees earliest (recip
                    # only), so the chain's head never waits on st drains
                    nc.tensor.matmul(
                        ps_l[:], lhsT=lhsT, rhs=ones_sb[:],
                        start=first, stop=last,
                    )
                    nc.tensor.matmul(
                        ps_o0[:], lhsT=lhsT, rhs=v_t[:, k % 4, 0:IB],
                        start=first, stop=last,
                    )
                    nc.tensor.matmul(
                        ps_o1[:], lhsT=lhsT, rhs=v_t[:, k % 4, IB:D],
                        start=first, stop=last,
                    )
                recip = wk.tile([P, 2], F32, tag="recip", bufs=2)
                nc.vector.reciprocal(recip[:], ps_l[:])
                st0 = wk.tile([P, IB], F32, tag="st0", bufs=2)
                st1 = wk.tile([P, IB], F32, tag="st1", bufs=2)
                # drain the two halves on different engines (DVE + ACT)
                nc.vector.tensor_scalar_mul(st0[:], ps_o0[:], recip[:, 0:1])
                nc.scalar.activation(st1[:], ps_o1[:],
                                     mybir.ActivationFunctionType.Copy,
                                     scale=recip[:, 0:1])
                nc.gpsimd.dma_start(out_d.ap()[i0:i0 + P, 0:IB], st0[:])
                nc.gpsimd.dma_start(out_d.ap()[i0:i0 + P, IB:D], st1[:])


def kernel(x: np.ndarray, Wq: np.ndarray, Wk: np.ndarray, Wv: np.ndarray) -> np.ndarray:
    if "nc" not in _CACHE:
        _CACHE["nc"] = _build()
    nc = _CACHE["nc"]

    x = np.ascontiguousarray(np.asarray(x, dtype=np.float32))
    Wq = np.ascontiguousarray(np.asarray(Wq, dtype=np.float32))
    Wk = np.ascontiguousarray(np.asarray(Wk, dtype=np.float32))
    Wv = np.ascontiguousarray(np.asarray(Wv, dtype=np.float32))

    in_maps = [
        {"x": x[c], "Wq": Wq, "Wk": Wk, "Wv": Wv} for c in range(B)
    ]
    res = run_bass_kernel_spmd(nc, in_maps, core_ids=list(range(B)))
    return np.stack([res.results[c]["out"] for c in range(B)], axis=0)


def _selftest():
    """Smoke test against a numpy fp64 reference on random data."""
    rng = np.random.default_rng(0)
    x = rng.standard_normal((B, S, D), dtype=np.float32)
    w = [rng.standard_normal((D, D), dtype=np.float32).astype(np.float32) / 32.0
         for _ in range(3)]
    out = kernel(x, *w)
    x64 = x.astype(np.float64)
    q, k, v = (x64 @ wi.astype(np.float64) for wi in w)
    s = np.einsum("bqe,bke->bqk", q, k) / 32.0
    mask = np.triu(np.ones((S, S), dtype=bool), k=1)
    s = np.where(mask[None], -np.inf, s)
    s -= s.max(-1, keepdims=True)
    p = np.exp(s)
    p /= p.sum(-1, keepdims=True)
    ref = np.einsum("bqk,bke->bqe", p, v)
    fro = np.linalg.norm(out - ref) / np.linalg.norm(ref)
    print(f"selftest rel err: {fro:.3e}")
    return fro


if __name__ == "__main__":
    _selftest()
